# revision 21
# baseline (speedup 1.0000x reference)
"""Trainium2 Bass kernel for an 8-layer weight-shared decoder stack (v3, fp16).

Model (see problem reference): h = emb[x]; 8x identical decoder layers
(LN -> single-head attn tiled 16x -> proj -> LN -> 4x FFN); fc to vocab.

Distribution over 8 NeuronCores:
  - tokens sharded 8-way (cores 0-3 <- batch 0, cores 4-7 <- batch 1;
    512 tokens per core); per-layer AllGather of K/V within each 4-core
    batch group;
  - final hidden AllGathered nowhere: fc computes local tokens x full
    vocab; host concatenates the token shards and casts fp16 -> fp32.

v3 structure (vs v2): fp16 residual stream; all biases/affines are zero
(asserted) and dropped; LN mean folded as a rank-1 matmul correction
(qkv) or by centering h (FFN); LN istd applied AFTER the matmuls
(relu(istd*x) = istd*relu(x) since istd>0 and c1=0), so the PE never
waits on the rsqrt chain; softmax denominator applied after the proj
matmul (column scaling commutes); LN stats accumulate interleaved with
the producer matmuls; attention score/AV matmuls run as row/col-tiled
pairs so the 64-wide ops fill the full 128-wide PE array.
"""
import numpy as np
from contextlib import ExitStack

import concourse.bass as bass
import concourse.tile as tile
from concourse import bacc, mybir
from concourse.bass_utils import run_bass_kernel_spmd
from concourse.masks import make_identity

dt = mybir.dt
AF = mybir.ActivationFunctionType
ALU = mybir.AluOpType

# model dims (hardcoded per the problem spec)
VOCAB, EMB, SEQ, STACK, N_HEADS, ATTN, BATCH = 32000, 1024, 2048, 8, 16, 64, 2
N_CORES = 8
T = (BATCH * SEQ) // N_CORES          # 512 tokens per core
GRP = 4                               # cores per batch group
GROUPS = [[0, 1, 2, 3], [4, 5, 6, 7]]
EC = EMB // 128                       # 8 emb chunks
KC = SEQ // 128                       # 16 key chunks (per batch)
HC = 4 * EMB // 128                   # 32 ffn hidden chunks
TC = T // 128                         # 4 local token chunks
VSH = VOCAB // N_CORES
VCC = 8
VCW = VSH // VCC                      # 500 cols per fc chunk
F32, I32 = dt.float32, dt.int32
MDT = dt.float16                      # matmul operand dtype
NDT = np.float16


def build_nc():
    nc = bacc.Bacc("TRN2", target_bir_lowering=False, debug=False,
                   enable_asserts=True, num_devices=N_CORES)

    # ---- I/O ----  (w1/w2/wfc are host-swizzled; see prepare_in_maps)
    emb = nc.dram_tensor("emb", [VOCAB, EMB], F32, kind="ExternalInput").ap()
    xi = nc.dram_tensor("xi", [T, 1], I32, kind="ExternalInput").ap()
    wvk = nc.dram_tensor("wvk", [EMB, 128], MDT, kind="ExternalInput").ap()
    wqq = nc.dram_tensor("wqq", [EMB, 128], MDT, kind="ExternalInput").ap()
    svk = nc.dram_tensor("svk", [1, 128], MDT, kind="ExternalInput").ap()
    sqq = nc.dram_tensor("sqq", [1, 128], MDT, kind="ExternalInput").ap()
    wd = nc.dram_tensor("wd", [ATTN, EMB], MDT, kind="ExternalInput").ap()
    w1 = nc.dram_tensor("w1", [HC, 128, EC * 128], MDT,
                        kind="ExternalInput").ap()          # [hc][p][ec*m]
    w2 = nc.dram_tensor("w2", [2, EC, 128, (HC // 2) * 128], MDT,
                        kind="ExternalInput").ap()          # [half][ec][p][j*m]
    wfc = nc.dram_tensor("wfc", [VOCAB // VCW, 128, EC * VCW], MDT,
                         kind="ExternalInput").ap()         # [vc][p][ec*n]
    mbias = nc.dram_tensor("mbias", [128, GRP], F32, kind="ExternalInput").ap()
    out = nc.dram_tensor("out", [T, VOCAB], MDT, kind="ExternalOutput").ap()

    with tile.TileContext(nc) as tc, ExitStack() as ctx:
        dram = ctx.enter_context(tc.tile_pool(name="dram", bufs=1, space="DRAM"))
        consts = ctx.enter_context(tc.tile_pool(name="consts", bufs=1))

        # ---- constants / small weights resident in SBUF ----
        ident = consts.tile([128, 128], F32, tag="ident")
        make_identity(nc, ident[:])
        identh = consts.tile([64, 64], MDT, tag="identh")
        nc.vector.tensor_copy(identh[:], ident[:64, :64])
        ones_f = consts.tile([128, 1], F32, tag="ones_f")
        nc.vector.memset(ones_f[:], 1.0)
        onesc = consts.tile([128, 1], MDT, tag="onesc")      # ones column
        nc.vector.tensor_copy(onesc[:], ones_f[:])
        ones_rowf = consts.tile([1, T], F32, tag="ones_rowf")
        nc.vector.memset(ones_rowf[:], 1.0)
        onesr = consts.tile([1, T], MDT, tag="onesr")        # ones row
        nc.vector.tensor_copy(onesr[:], ones_rowf[:])
        zbias = consts.tile([128, 1], F32, tag="zbias")
        nc.vector.memset(zbias[:], 0.0)
        mbias_t = consts.tile([128, GRP], F32, tag="mbias")
        nc.sync.dma_start(mbias_t[:], mbias)

        wvk_t = consts.tile([128, EC * 128], MDT, tag="wvk")
        nc.sync.dma_start(
            wvk_t.rearrange("p (ec a) -> p ec a", ec=EC),
            wvk.rearrange("(ec p) a -> p ec a", p=128))
        wqq_t = consts.tile([128, EC * 128], MDT, tag="wqq")
        nc.sync.dma_start(
            wqq_t.rearrange("p (ec a) -> p ec a", ec=EC),
            wqq.rearrange("(ec p) a -> p ec a", p=128))
        svk_t = consts.tile([1, 128], MDT, tag="svk")
        nc.sync.dma_start(svk_t[:], svk)
        sqq_t = consts.tile([1, 128], MDT, tag="sqq")
        nc.sync.dma_start(sqq_t[:], sqq)
        wd_t = consts.tile([ATTN, EMB], MDT, tag="wd")
        nc.sync.dma_start(wd_t[:], wd)

        # residual stream, fp16, [emb-part, token-free]; alive through fc
        hfp = ctx.enter_context(tc.tile_pool(name="hfp", bufs=1))
        h16 = hfp.tile([128, EC * T], MDT, tag="h16")

        def hcol(ec):
            return h16[:, ec * T:(ec + 1) * T]

        # ================= phase 1: embed + decoder stack =================
        with ExitStack() as lctx:
            ps_mm = lctx.enter_context(
                tc.tile_pool(name="ps_mm", bufs=3, space="PSUM"))
            ps_uv = lctx.enter_context(
                tc.tile_pool(name="ps_uv", bufs=2, space="PSUM"))
            ps_x = lctx.enter_context(
                tc.tile_pool(name="ps_x", bufs=2, space="PSUM"))
            ps_rows = lctx.enter_context(
                tc.tile_pool(name="ps_rows", bufs=1, space="PSUM"))
            lay = lctx.enter_context(tc.tile_pool(name="lay", bufs=2))
            scr = lctx.enter_context(tc.tile_pool(name="scratch", bufs=2))
            sqp = lctx.enter_context(tc.tile_pool(name="sqp", bufs=2))
            bcp = lctx.enter_context(tc.tile_pool(name="bcp", bufs=3))
            a1p = lctx.enter_context(tc.tile_pool(name="a1p", bufs=1))
            w1p = lctx.enter_context(tc.tile_pool(name="w1p", bufs=4))
            w2p = lctx.enter_context(tc.tile_pool(name="w2p", bufs=2))
            etp = lctx.enter_context(tc.tile_pool(name="etp", bufs=4))
            kvp = lctx.enter_context(tc.tile_pool(name="kvp", bufs=2))
            rows = lctx.enter_context(tc.tile_pool(name="rows", bufs=8))
            embp = lctx.enter_context(tc.tile_pool(name="embp", bufs=2))

            # ---- embedding gather + transpose (fp16 out) ----
            with nc.named_scope("embed"):
                for tk in range(TC):
                    idx_t = embp.tile([128, 1], I32, tag="idx")
                    nc.sync.dma_start(idx_t[:], xi[tk * 128:(tk + 1) * 128, :])
                    gat = embp.tile([128, EMB], F32, tag="gat")
                    nc.gpsimd.indirect_dma_start(
                        out=gat[:], out_offset=None, in_=emb,
                        in_offset=bass.IndirectOffsetOnAxis(ap=idx_t[:, :1], axis=0))
                    for ec in range(EC):
                        tr_ps = ps_mm.tile([128, 128], F32, tag="mm")
                        nc.tensor.transpose(
                            tr_ps[:], gat[:, ec * 128:(ec + 1) * 128], ident[:])
                        nc.vector.tensor_copy(
                            h16[:, ec * T + tk * 128: ec * T + (tk + 1) * 128],
                            tr_ps[:])

            def emit_stats(st, ec, first, last):
                """Accumulate per-token sum (row 0) and sum-of-squares
                (row 32) of h chunk ec into the stats psum tile."""
                nc.tensor.matmul(st[0:1, :], onesc[:], hcol(ec),
                                 start=first, stop=last)
                hsq = sqp.tile([128, T], MDT, tag="hsq")
                nc.scalar.activation(hsq[:], hcol(ec), AF.Square)
                nc.tensor.matmul(st[32:33, :], onesc[:], hsq[:],
                                 start=first, stop=last)

            def ln_rows(st):
                """From stats psum -> (nmu16 row, istd16 row, istd_b f32).
                Returns (nmu16, istd16). The rsqrt runs on DVE (bit-trick
                seed + 2 Newton steps), off the PE critical path."""
                nmu = rows.tile([1, T], F32, tag="r1")
                nc.vector.tensor_scalar(nmu[:], st[0:1, :], -1.0 / EMB, None,
                                        op0=ALU.mult)
                nmu16 = rows.tile([1, T], MDT, tag="r1h")
                nc.vector.tensor_copy(nmu16[:], nmu[:])
                var = rows.tile([1, T], F32, tag="r1")
                nc.vector.tensor_scalar(var[:], st[32:33, :], 1.0 / EMB, 1e-5,
                                        op0=ALU.mult, op1=ALU.add)
                musq = rows.tile([1, T], F32, tag="r1")
                nc.vector.tensor_tensor(musq[:], nmu[:], nmu[:], op=ALU.mult)
                nc.vector.tensor_tensor(var[:], var[:], musq[:], op=ALU.subtract)
                y = rows.tile([1, T], I32, tag="r1i")
                nc.vector.tensor_scalar(y[:], var[:].bitcast(I32), 1, None,
                                        op0=ALU.logical_shift_right)
                nc.vector.tensor_scalar(y[:], y[:], -1, 0x5f3759df,
                                        op0=ALU.mult, op1=ALU.add)
                yf = y[:].bitcast(F32)
                for _ in range(1):
                    a = rows.tile([1, T], F32, tag="r1")
                    nc.vector.tensor_tensor(a[:], yf, yf, op=ALU.mult)
                    nc.vector.tensor_tensor(a[:], a[:], var[:], op=ALU.mult)
                    nc.vector.tensor_scalar(a[:], a[:], -0.5, 1.5,
                                            op0=ALU.mult, op1=ALU.add)
                    nc.vector.tensor_tensor(yf, yf, a[:], op=ALU.mult)
                istd16 = rows.tile([1, T], MDT, tag="r1h")
                nc.vector.tensor_copy(istd16[:], yf)
                return nmu16, istd16

            def bcast(row16, out_dt, tag):
                """Broadcast a [1,T] fp16 row to all 128 partitions."""
                b_ps = ps_x.tile([128, T], F32, tag="x", name=f"b_{tag}")
                nc.tensor.matmul(b_ps[:], onesr[:, :128], row16[:],
                                 start=True, stop=True)
                b_sb = bcp.tile([128, T], out_dt, tag="bc", name=f"bc_{tag}")
                nc.vector.tensor_copy(b_sb[:], b_ps[:])
                return b_sb

            # initial LN1 stats (for layer 0)
            st = ps_rows.tile([128, T], F32, tag="st")
            for ec in range(EC):
                emit_stats(st, ec, ec == 0, ec == EC - 1)

            scale = float(ATTN) ** -0.5
            for layer in range(STACK):
                with nc.named_scope(f"L{layer}"):
                    # ---- LN1 rows + u matmuls (no LN wait on PE) ----
                    nmu16, istd16 = ln_rows(st)
                    kv_ps = ps_uv.tile([128, T], F32, tag="uv")
                    qq_ps = ps_uv.tile([128, T], F32, tag="uv")
                    for ec in range(EC):
                        nc.tensor.matmul(kv_ps[:],
                                         wvk_t[:, ec * 128:(ec + 1) * 128],
                                         hcol(ec), start=(ec == 0), stop=False)
                    nc.tensor.matmul(kv_ps[:], svk_t[:], nmu16[:],
                                     start=False, stop=True)
                    for ec in range(EC):
                        nc.tensor.matmul(qq_ps[:],
                                         wqq_t[:, ec * 128:(ec + 1) * 128],
                                         hcol(ec), start=(ec == 0), stop=False)
                    nc.tensor.matmul(qq_ps[:], sqq_t[:], nmu16[:],
                                     start=False, stop=True)
                    istd1_b = bcast(istd16, F32, "i1")
                    # vk: v on partitions 0:64, k on 64:128; qq: q on both
                    vk_t = lay.tile([128, T], MDT, tag="vk")
                    nc.vector.tensor_tensor(vk_t[:], kv_ps[:], istd1_b[:],
                                            op=ALU.mult)
                    qq_t = lay.tile([128, T], MDT, tag="qq")
                    nc.vector.tensor_tensor(qq_t[:], qq_ps[:], istd1_b[:],
                                            op=ALU.mult)

                    # ---- local v -> token-major; stage k|v; gather ----
                    v_loc = lay.tile([128, TC * ATTN], MDT, tag="vloc")
                    for tk in range(TC):
                        tp = ps_x.tile([128, 128], MDT, tag="x")
                        nc.tensor.transpose(
                            tp[:128, :ATTN],
                            vk_t[:ATTN, tk * 128:(tk + 1) * 128], identh[:])
                        nc.vector.tensor_copy(
                            v_loc[:, tk * ATTN:(tk + 1) * ATTN],
                            tp[:128, :ATTN])
                    kv_loc = dram.tile([2 * ATTN * T], MDT, tag="kv_loc")
                    nc.sync.dma_start(
                        kv_loc[0:ATTN * T].rearrange("(a t) -> a t", a=ATTN),
                        vk_t[64:128, :])
                    nc.sync.dma_start(
                        kv_loc[ATTN * T:].rearrange("(p c) -> p c", p=128),
                        v_loc[:])
                    kv_g = dram.tile([GRP, 2 * ATTN * T], MDT, tag="kv_g")
                    nc.gpsimd.collective_compute(
                        "AllGather", ALU.bypass, replica_groups=GROUPS,
                        ins=[kv_loc.opt()], outs=[kv_g.opt()])

                    # ---- attention ----
                    den = ps_rows.tile([128, T], F32, tag="st", name="den")
                    hav = ps_x.tile([128, T], F32, tag="x", name="hav")
                    n_e = 0          # den accumulation counter (20 total)
                    n_av = [0, 0]    # AV chain counters per psum half

                    def av_den(e_t, vsrc, kc):
                        nonlocal n_e
                        half = kc % 2
                        nc.tensor.matmul(den[0:1, :], onesc[:], e_t[:],
                                         start=(n_e == 0), stop=(n_e == 19))
                        n_e += 1
                        nc.tensor.matmul(
                            hav[64 * half:64 * half + 64, :],
                            vsrc, e_t[:],
                            start=(n_av[half] == 0), stop=(n_av[half] == 9))
                        n_av[half] += 1

                    # local quarter first (overlaps the gather)
                    for lk in range(TC):
                        s_ps = ps_mm.tile([128, T], F32, tag="mm")
                        nc.tensor.matmul(s_ps[:],
                                         vk_t[64:128, lk * 128:(lk + 1) * 128],
                                         qq_t[64:128, :], start=True, stop=True)
                        e_t = etp.tile([128, T], MDT, tag="eT")
                        nc.scalar.activation(e_t[:], s_ps[:], AF.Exp,
                                             scale=scale, bias=zbias[:, :1])
                        av_den(e_t, v_loc[:, lk * ATTN:(lk + 1) * ATTN], lk)

                    # gathered full sequence, paired layouts
                    khat = kvp.tile([128, (KC // 2) * 128], MDT, tag="khat")
                    vtm = kvp.tile([128, KC * ATTN], MDT, tag="vtm")
                    for r in range(GRP):
                        src_k = kv_g[r, 0:ATTN * T].rearrange(
                            "(a e two t) -> two a e t", a=ATTN, e=2, two=2,
                            t=128)
                        for two in range(2):
                            dst = khat[64 * two:64 * two + 64,
                                       (2 * r) * 128:(2 * r + 2) * 128]
                            nc.sync.dma_start(
                                dst.rearrange("p (e t) -> p e t", e=2),
                                src_k[two, :, :, :])
                        nc.sync.dma_start(
                            vtm[:, r * TC * ATTN:(r + 1) * TC * ATTN]
                            .rearrange("p (c a) -> p c a", c=TC),
                            kv_g[r, ATTN * T:]
                            .rearrange("(p c a) -> p c a", p=128, c=TC))
                    for pc in range(KC // 2):
                        r = pc // 2
                        sA = ps_mm.tile([128, T], F32, tag="mm")
                        nc.tensor.matmul(sA[:],
                                         khat[0:64, pc * 128:(pc + 1) * 128],
                                         qq_t[0:64, :], start=True, stop=True)
                        sB = ps_mm.tile([128, T], F32, tag="mm")
                        nc.tensor.matmul(sB[:],
                                         khat[64:128, pc * 128:(pc + 1) * 128],
                                         qq_t[64:128, :], start=True, stop=True)
                        eA = etp.tile([128, T], MDT, tag="eT")
                        nc.scalar.activation(eA[:], sA[:], AF.Exp,
                                             scale=scale, bias=mbias_t[:, r:r + 1])
                        eB = etp.tile([128, T], MDT, tag="eT")
                        nc.scalar.activation(eB[:], sB[:], AF.Exp,
                                             scale=scale, bias=mbias_t[:, r:r + 1])
                        kc = 2 * pc
                        av_den(eA, vtm[:, kc * ATTN:(kc + 1) * ATTN], kc)
                        av_den(eB, vtm[:, (kc + 1) * ATTN:(kc + 2) * ATTN],
                               kc + 1)

                    # 1/den (single DVE op), fp16 row, broadcast
                    dsb = rows.tile([1, T], F32, tag="r1")
                    nc.vector.tensor_copy(dsb[:], den[0:1, :])
                    rrow = rows.tile([1, T], F32, tag="r1")
                    nc.vector.reciprocal(rrow[:], dsb[:])
                    rrow16 = rows.tile([1, T], MDT, tag="r1h")
                    nc.vector.tensor_copy(rrow16[:], rrow[:])
                    rb_b = bcast(rrow16, F32, "rb")
                    havB = lay.tile([ATTN, T], F32, tag="havB")
                    nc.scalar.activation(havB[:], hav[64:128, :], AF.Identity,
                                         bias=zbias[:ATTN, :1])
                    headT = lay.tile([ATTN, T], MDT, tag="headT")
                    nc.vector.tensor_tensor(headT[:], hav[0:64, :],
                                            havB[:], op=ALU.add)

                    # ---- proj + residual + LN2 stats (interleaved) ----
                    st = ps_rows.tile([128, T], F32, tag="st")
                    for ec in range(EC):
                        p_ps = ps_mm.tile([128, T], F32, tag="mm")
                        nc.tensor.matmul(p_ps[:], wd_t[:, ec * 128:(ec + 1) * 128],
                                         headT[:], start=True, stop=True)
                        t_sb = sqp.tile([128, T], MDT, tag="tsb")
                        nc.vector.tensor_tensor(t_sb[:], p_ps[:], rb_b[:],
                                                op=ALU.mult)
                        nc.gpsimd.tensor_tensor(hcol(ec), hcol(ec), t_sb[:],
                                                op=ALU.add)
                        emit_stats(st, ec, ec == 0, ec == EC - 1)

                    # ---- LN2 rows; centered h (fp16); FFN unscaled ----
                    nmu16, istd16 = ln_rows(st)
                    nmu2_b = bcast(nmu16, MDT, "m2")
                    hcen = scr.tile([128, EC * T], MDT, tag="hcen")
                    for ec in range(EC):
                        nc.gpsimd.tensor_tensor(
                            hcen[:, ec * T:(ec + 1) * T], hcol(ec), nmu2_b[:],
                            op=ALU.add)
                    istd2_b = bcast(istd16, F32, "i2")
                    for half in range(2):
                        a1 = a1p.tile([128, (HC // 2) * T], MDT, tag="a1")
                        for j in range(HC // 2):
                            hc = half * (HC // 2) + j
                            w1_t = w1p.tile([128, EC * 128], MDT, tag="w1")
                            nc.sync.dma_start(w1_t[:], w1[hc])
                            f_ps = ps_mm.tile([128, T], F32, tag="mm")
                            for ec in range(EC):
                                nc.tensor.matmul(
                                    f_ps[:], w1_t[:, ec * 128:(ec + 1) * 128],
                                    hcen[:, ec * T:(ec + 1) * T],
                                    start=(ec == 0), stop=(ec == EC - 1))
                            nc.scalar.activation(a1[:, j * T:(j + 1) * T], f_ps[:],
                                                 AF.Relu, bias=zbias[:, :1])
                        last_half = half == 1
                        if last_half and layer < STACK - 1:
                            st = ps_rows.tile([128, T], F32, tag="st")
                        for ec in range(EC):
                            w2_t = w2p.tile([128, (HC // 2) * 128], MDT, tag="w2")
                            nc.sync.dma_start(w2_t[:], w2[half, ec])
                            g_ps = ps_mm.tile([128, T], F32, tag="mm")
                            for j in range(HC // 2):
                                nc.tensor.matmul(
                                    g_ps[:], w2_t[:, j * 128:(j + 1) * 128],
                                    a1[:, j * T:(j + 1) * T],
                                    start=(j == 0), stop=(j == HC // 2 - 1))
                            t_sb = sqp.tile([128, T], MDT, tag="tsb")
                            nc.vector.tensor_tensor(t_sb[:], g_ps[:],
                                                    istd2_b[:], op=ALU.mult)
                            nc.gpsimd.tensor_tensor(hcol(ec), hcol(ec), t_sb[:],
                                                    op=ALU.add)
                            if last_half and layer < STACK - 1:
                                emit_stats(st, ec, ec == 0, ec == EC - 1)

        # ======= phase 2: fc, local tokens x full vocab (no collective) =======
        # Groups of GV=4 vocab chunks stay SBUF-resident; within a group the
        # token-chunk stationary is reused across 4 PSUM banks (one per vocab
        # chunk) so LDWEIGHTS amortizes 4x. Output written fp16 (host casts).
        with nc.named_scope("fc"):
            GV = 4
            NVC = VOCAB // VCW
            with tc.tile_pool(name="wfcp", bufs=2) as wfcp, \
                 tc.tile_pool(name="outp", bufs=8) as outp, \
                 tc.tile_pool(name="ps_fc", bufs=2, space="PSUM") as ps_fc:
                for g in range(NVC // GV):
                    wg = wfcp.tile([128, GV * EC * VCW], MDT, tag="wfc")
                    for b in range(GV):
                        nc.sync.dma_start(
                            wg[:, b * EC * VCW:(b + 1) * EC * VCW],
                            wfc[g * GV + b])
                    for tcg in range(TC):
                        # 4 bank-aligned psum slots (512-col padded)
                        o_ps = ps_fc.tile([128, GV, 512], F32, tag="fc")
                        for ec in range(EC):
                            hs = h16[:, ec * T + tcg * 128:
                                     ec * T + (tcg + 1) * 128]
                            for b in range(GV):
                                nc.tensor.matmul(
                                    o_ps[:, b, :VCW], hs,
                                    wg[:, (b * EC + ec) * VCW:
                                       (b * EC + ec + 1) * VCW],
                                    start=(ec == 0), stop=(ec == EC - 1))
                        for b in range(GV):
                            o_sb = outp.tile([128, VCW], MDT, tag="osb")
                            nc.scalar.activation(o_sb[:], o_ps[:, b, :VCW],
                                                 AF.Identity,
                                                 bias=zbias[:, :1])
                            nc.sync.dma_start(
                                out[tcg * 128:(tcg + 1) * 128,
                                    (g * GV + b) * VCW:
                                    (g * GV + b + 1) * VCW], o_sb[:])

    nc.compile()
    return nc


_NC_CACHE = None


def _get_nc():
    global _NC_CACHE
    if _NC_CACHE is None:
        _NC_CACHE = build_nc()
    return _NC_CACHE


def prepare_in_maps(inputs):
    f32 = np.float32
    x = np.asarray(inputs["x"]).reshape(-1).astype(np.int32)
    emb = np.ascontiguousarray(np.asarray(inputs["emb"], f32))
    g1 = np.asarray(inputs["g1"], f32)
    g2 = np.asarray(inputs["g2"], f32)
    # all biases of this model are zero (and beta@W folds are then zero);
    # the kernel relies on that, so assert it here.
    for k in ("bq", "bk", "bv", "bd", "c1", "c2", "bfc", "beta1", "beta2"):
        assert np.abs(np.asarray(inputs[k], f32)).max() == 0.0, f"{k} nonzero"
    # fold LN1 affine into qkv projections; pack [v|k] and [q|q]
    wq_f = (g1[:, None] * np.asarray(inputs["Wq"], f32)).astype(NDT)
    wk_f = (g1[:, None] * np.asarray(inputs["Wk"], f32)).astype(NDT)
    wv_f = (g1[:, None] * np.asarray(inputs["Wv"], f32)).astype(NDT)
    wvk = np.ascontiguousarray(np.concatenate([wv_f, wk_f], axis=1))
    wqq = np.ascontiguousarray(np.concatenate([wq_f, wq_f], axis=1))
    # rank-1 mean-correction rows: colsums of the folded fp16 weights
    svk = np.ascontiguousarray(
        wvk.astype(f32).sum(0, keepdims=True).astype(NDT))
    sqq = np.ascontiguousarray(
        wqq.astype(f32).sum(0, keepdims=True).astype(NDT))
    # tile(head, 16) @ Wd == head @ (sum of the 16 row-blocks of Wd)
    Wd_sum = np.asarray(inputs["Wd"], f32).reshape(N_HEADS, ATTN, EMB).sum(0)
    wd_h = np.ascontiguousarray(Wd_sum.astype(NDT))
    # fold LN2 affine into W1; swizzle to [hc][p][ec*128]
    W1 = np.asarray(inputs["W1"], f32)
    w1_f = (g2[:, None] * W1).astype(NDT)                    # [1024, 4096]
    w1_sw = np.ascontiguousarray(
        w1_f.reshape(EC, 128, HC, 128).transpose(2, 1, 0, 3)
        .reshape(HC, 128, EC * 128))
    # W2 swizzle to [half][ec][p][j*128]
    W2 = np.asarray(inputs["W2"], f32).astype(NDT)           # [4096, 1024]
    w2_sw = np.ascontiguousarray(
        W2.reshape(2, HC // 2, 128, EC, 128).transpose(0, 3, 2, 1, 4)
        .reshape(2, EC, 128, (HC // 2) * 128))
    Wfc = np.asarray(inputs["Wfc"], f32)

    NVC = VOCAB // VCW
    wfc_sw = np.ascontiguousarray(
        Wfc.astype(NDT).reshape(EC, 128, NVC, VCW).transpose(2, 1, 0, 3)
        .reshape(NVC, 128, EC * VCW))
    in_maps = []
    for c in range(N_CORES):
        mb = np.zeros((128, GRP), np.float32)
        mb[:, c % GRP] = -1e4
        in_maps.append(dict(
            emb=emb,
            xi=np.ascontiguousarray(x[c * T:(c + 1) * T, None]),
            wvk=wvk, wqq=wqq, svk=svk, sqq=sqq,
            wd=wd_h, w1=w1_sw, w2=w2_sw,
            wfc=wfc_sw, mbias=mb,
        ))
    return in_maps


def kernel(**inputs) -> np.ndarray:
    nc = _get_nc()
    in_maps = prepare_in_maps(inputs)
    r = run_bass_kernel_spmd(nc, in_maps, core_ids=list(range(N_CORES)))
    logits = np.concatenate([r.results[c]["out"] for c in range(N_CORES)], axis=0)
    return logits.reshape(BATCH, SEQ, VOCAB).astype(np.float32)


# revision 25
# speedup vs baseline: 1.0430x; 1.0430x over previous
"""Trainium2 Bass kernel for an 8-layer weight-shared decoder stack (v3, fp16).

Model (see problem reference): h = emb[x]; 8x identical decoder layers
(LN -> single-head attn tiled 16x -> proj -> LN -> 4x FFN); fc to vocab.

Distribution over 8 NeuronCores:
  - tokens sharded 8-way (cores 0-3 <- batch 0, cores 4-7 <- batch 1;
    512 tokens per core); per-layer AllGather of K/V within each 4-core
    batch group;
  - final hidden AllGathered nowhere: fc computes local tokens x full
    vocab; host concatenates the token shards and casts fp16 -> fp32.

v3 structure (vs v2): fp16 residual stream; all biases/affines are zero
(asserted) and dropped; LN mean folded as a rank-1 matmul correction
(qkv) or by centering h (FFN); LN istd applied AFTER the matmuls
(relu(istd*x) = istd*relu(x) since istd>0 and c1=0), so the PE never
waits on the rsqrt chain; softmax denominator applied after the proj
matmul (column scaling commutes); LN stats accumulate interleaved with
the producer matmuls; attention score/AV matmuls run as row/col-tiled
pairs so the 64-wide ops fill the full 128-wide PE array.
"""
import numpy as np
from contextlib import ExitStack

import concourse.bass as bass
import concourse.tile as tile
from concourse import bacc, mybir
from concourse.bass_utils import run_bass_kernel_spmd
from concourse.masks import make_identity

dt = mybir.dt
AF = mybir.ActivationFunctionType
ALU = mybir.AluOpType

# model dims (hardcoded per the problem spec)
VOCAB, EMB, SEQ, STACK, N_HEADS, ATTN, BATCH = 32000, 1024, 2048, 8, 16, 64, 2
N_CORES = 8
T = (BATCH * SEQ) // N_CORES          # 512 tokens per core
GRP = 4                               # cores per batch group
GROUPS = [[0, 1, 2, 3], [4, 5, 6, 7]]
EC = EMB // 128                       # 8 emb chunks
KC = SEQ // 128                       # 16 key chunks (per batch)
HC = 4 * EMB // 128                   # 32 ffn hidden chunks
TC = T // 128                         # 4 local token chunks
VSH = VOCAB // N_CORES
VCC = 8
VCW = VSH // VCC                      # 500 cols per fc chunk
F32, I32 = dt.float32, dt.int32
MDT = dt.float16                      # matmul operand dtype
NDT = np.float16


def build_nc():
    nc = bacc.Bacc("TRN2", target_bir_lowering=False, debug=False,
                   enable_asserts=True, num_devices=N_CORES)

    # ---- I/O ----  (w1/w2/wfc are host-swizzled; see prepare_in_maps)
    emb = nc.dram_tensor("emb", [VOCAB, EMB], F32, kind="ExternalInput").ap()
    xi = nc.dram_tensor("xi", [T, 1], I32, kind="ExternalInput").ap()
    wvk = nc.dram_tensor("wvk", [EMB, 128], MDT, kind="ExternalInput").ap()
    wqq = nc.dram_tensor("wqq", [EMB, 128], MDT, kind="ExternalInput").ap()
    svk = nc.dram_tensor("svk", [1, 128], MDT, kind="ExternalInput").ap()
    sqq = nc.dram_tensor("sqq", [1, 128], MDT, kind="ExternalInput").ap()
    wd = nc.dram_tensor("wd", [ATTN, EMB], MDT, kind="ExternalInput").ap()
    w1 = nc.dram_tensor("w1", [HC, 128, EC * 128], MDT,
                        kind="ExternalInput").ap()          # [hc][p][ec*m]
    w2 = nc.dram_tensor("w2", [2, EC, 128, (HC // 2) * 128], MDT,
                        kind="ExternalInput").ap()          # [half][ec][p][j*m]
    wfc = nc.dram_tensor("wfc", [VOCAB // VCW, 128, EC * VCW], MDT,
                         kind="ExternalInput").ap()         # [vc][p][ec*n]
    mbias = nc.dram_tensor("mbias", [128, GRP], F32, kind="ExternalInput").ap()
    out = nc.dram_tensor("out", [T, VOCAB], MDT, kind="ExternalOutput").ap()

    with tile.TileContext(nc) as tc, ExitStack() as ctx:
        dram = ctx.enter_context(tc.tile_pool(name="dram", bufs=1, space="DRAM"))
        consts = ctx.enter_context(tc.tile_pool(name="consts", bufs=1))

        # ---- constants / small weights resident in SBUF ----
        ident = consts.tile([128, 128], F32, tag="ident")
        make_identity(nc, ident[:])
        identh = consts.tile([64, 64], MDT, tag="identh")
        nc.vector.tensor_copy(identh[:], ident[:64, :64])
        ones_f = consts.tile([128, 1], F32, tag="ones_f")
        nc.vector.memset(ones_f[:], 1.0)
        onesc = consts.tile([128, 1], MDT, tag="onesc")      # ones column
        nc.vector.tensor_copy(onesc[:], ones_f[:])
        ones_rowf = consts.tile([1, T], F32, tag="ones_rowf")
        nc.vector.memset(ones_rowf[:], 1.0)
        onesr = consts.tile([1, T], MDT, tag="onesr")        # ones row
        nc.vector.tensor_copy(onesr[:], ones_rowf[:])
        zbias = consts.tile([128, 1], F32, tag="zbias")
        nc.vector.memset(zbias[:], 0.0)
        mbias_t = consts.tile([128, GRP], F32, tag="mbias")
        nc.sync.dma_start(mbias_t[:], mbias)

        wvk_t = consts.tile([128, EC * 128], MDT, tag="wvk")
        nc.sync.dma_start(
            wvk_t.rearrange("p (ec a) -> p ec a", ec=EC),
            wvk.rearrange("(ec p) a -> p ec a", p=128))
        wqq_t = consts.tile([128, EC * 128], MDT, tag="wqq")
        nc.sync.dma_start(
            wqq_t.rearrange("p (ec a) -> p ec a", ec=EC),
            wqq.rearrange("(ec p) a -> p ec a", p=128))
        svk_t = consts.tile([1, 128], MDT, tag="svk")
        nc.sync.dma_start(svk_t[:], svk)
        sqq_t = consts.tile([1, 128], MDT, tag="sqq")
        nc.sync.dma_start(sqq_t[:], sqq)
        wd_t = consts.tile([ATTN, EMB], MDT, tag="wd")
        nc.sync.dma_start(wd_t[:], wd)

        # residual stream, fp16, [emb-part, token-free]; alive through fc
        hfp = ctx.enter_context(tc.tile_pool(name="hfp", bufs=1))
        h16 = hfp.tile([128, EC * T], MDT, tag="h16")

        def hcol(ec):
            return h16[:, ec * T:(ec + 1) * T]

        # ================= phase 1: embed + decoder stack =================
        with ExitStack() as lctx:
            ps_mm = lctx.enter_context(
                tc.tile_pool(name="ps_mm", bufs=3, space="PSUM"))
            ps_uv = lctx.enter_context(
                tc.tile_pool(name="ps_uv", bufs=2, space="PSUM"))
            ps_x = lctx.enter_context(
                tc.tile_pool(name="ps_x", bufs=2, space="PSUM"))
            ps_rows = lctx.enter_context(
                tc.tile_pool(name="ps_rows", bufs=1, space="PSUM"))
            lay = lctx.enter_context(tc.tile_pool(name="lay", bufs=2))
            scr = lctx.enter_context(tc.tile_pool(name="scratch", bufs=2))
            sqp = lctx.enter_context(tc.tile_pool(name="sqp", bufs=2))
            bcp = lctx.enter_context(tc.tile_pool(name="bcp", bufs=3))
            a1p = lctx.enter_context(tc.tile_pool(name="a1p", bufs=1))
            w1p = lctx.enter_context(tc.tile_pool(name="w1p", bufs=4))
            w2p = lctx.enter_context(tc.tile_pool(name="w2p", bufs=2))
            etp = lctx.enter_context(tc.tile_pool(name="etp", bufs=4))
            kvp = lctx.enter_context(tc.tile_pool(name="kvp", bufs=2))
            rows = lctx.enter_context(tc.tile_pool(name="rows", bufs=8))
            embp = lctx.enter_context(tc.tile_pool(name="embp", bufs=2))

            # ---- embedding gather + transpose (fp16 out) ----
            with nc.named_scope("embed"):
                for tk in range(TC):
                    idx_t = embp.tile([128, 1], I32, tag="idx")
                    nc.sync.dma_start(idx_t[:], xi[tk * 128:(tk + 1) * 128, :])
                    gat = embp.tile([128, EMB], F32, tag="gat")
                    nc.gpsimd.indirect_dma_start(
                        out=gat[:], out_offset=None, in_=emb,
                        in_offset=bass.IndirectOffsetOnAxis(ap=idx_t[:, :1], axis=0))
                    for ec in range(EC):
                        tr_ps = ps_mm.tile([128, 128], F32, tag="mm")
                        nc.tensor.transpose(
                            tr_ps[:], gat[:, ec * 128:(ec + 1) * 128], ident[:])
                        nc.vector.tensor_copy(
                            h16[:, ec * T + tk * 128: ec * T + (tk + 1) * 128],
                            tr_ps[:])

            def emit_stats(st, ec, first, last):
                """Accumulate per-token sum (row 0) and sum-of-squares
                (row 32) of h chunk ec into the stats psum tile."""
                nc.tensor.matmul(st[0:1, :], onesc[:], hcol(ec),
                                 start=first, stop=last)
                hsq = sqp.tile([128, T], MDT, tag="hsq")
                nc.scalar.activation(hsq[:], hcol(ec), AF.Square)
                nc.tensor.matmul(st[32:33, :], onesc[:], hsq[:],
                                 start=first, stop=last)

            def ln_rows(st):
                """From stats psum -> (nmu16 row, istd16 row, istd_b f32).
                Returns (nmu16, istd16). The rsqrt runs on DVE (bit-trick
                seed + 2 Newton steps), off the PE critical path."""
                nmu = rows.tile([1, T], F32, tag="r1")
                nc.vector.tensor_scalar(nmu[:], st[0:1, :], -1.0 / EMB, None,
                                        op0=ALU.mult)
                nmu16 = rows.tile([1, T], MDT, tag="r1h")
                nc.vector.tensor_copy(nmu16[:], nmu[:])
                var = rows.tile([1, T], F32, tag="r1")
                nc.vector.tensor_scalar(var[:], st[32:33, :], 1.0 / EMB, 1e-5,
                                        op0=ALU.mult, op1=ALU.add)
                musq = rows.tile([1, T], F32, tag="r1")
                nc.vector.tensor_tensor(musq[:], nmu[:], nmu[:], op=ALU.mult)
                nc.vector.tensor_tensor(var[:], var[:], musq[:], op=ALU.subtract)
                y = rows.tile([1, T], I32, tag="r1i")
                nc.vector.tensor_scalar(y[:], var[:].bitcast(I32), 1, None,
                                        op0=ALU.logical_shift_right)
                nc.vector.tensor_scalar(y[:], y[:], -1, 0x5f3759df,
                                        op0=ALU.mult, op1=ALU.add)
                yf = y[:].bitcast(F32)
                for _ in range(2):
                    a = rows.tile([1, T], F32, tag="r1")
                    nc.vector.tensor_tensor(a[:], yf, yf, op=ALU.mult)
                    nc.vector.tensor_tensor(a[:], a[:], var[:], op=ALU.mult)
                    nc.vector.tensor_scalar(a[:], a[:], -0.5, 1.5,
                                            op0=ALU.mult, op1=ALU.add)
                    nc.vector.tensor_tensor(yf, yf, a[:], op=ALU.mult)
                istd16 = rows.tile([1, T], MDT, tag="r1h")
                nc.vector.tensor_copy(istd16[:], yf)
                return nmu16, istd16

            def bcast(row16, out_dt, tag):
                """Broadcast a [1,T] fp16 row to all 128 partitions."""
                b_ps = ps_x.tile([128, T], F32, tag="x", name=f"b_{tag}")
                nc.tensor.matmul(b_ps[:], onesr[:, :128], row16[:],
                                 start=True, stop=True)
                b_sb = bcp.tile([128, T], out_dt, tag="bc", name=f"bc_{tag}")
                nc.vector.tensor_copy(b_sb[:], b_ps[:])
                return b_sb

            # initial LN1 stats (for layer 0)
            st = ps_rows.tile([128, T], F32, tag="st")
            for ec in range(EC):
                emit_stats(st, ec, ec == 0, ec == EC - 1)

            scale = float(ATTN) ** -0.5
            for layer in range(STACK):
                with nc.named_scope(f"L{layer}"):
                    # ---- LN1 rows + u matmuls (no LN wait on PE) ----
                    nmu16, istd16 = ln_rows(st)
                    kv_ps = ps_uv.tile([128, T], F32, tag="uv")
                    qq_ps = ps_uv.tile([128, T], F32, tag="uv")
                    for ec in range(EC):
                        nc.tensor.matmul(kv_ps[:],
                                         wvk_t[:, ec * 128:(ec + 1) * 128],
                                         hcol(ec), start=(ec == 0), stop=False)
                    nc.tensor.matmul(kv_ps[:], svk_t[:], nmu16[:],
                                     start=False, stop=True)
                    for ec in range(EC):
                        nc.tensor.matmul(qq_ps[:],
                                         wqq_t[:, ec * 128:(ec + 1) * 128],
                                         hcol(ec), start=(ec == 0), stop=False)
                    nc.tensor.matmul(qq_ps[:], sqq_t[:], nmu16[:],
                                     start=False, stop=True)
                    istd1_b = bcast(istd16, F32, "i1")
                    # vk: v on partitions 0:64, k on 64:128; qq: q on both
                    vk_t = lay.tile([128, T], MDT, tag="vk")
                    nc.vector.tensor_tensor(vk_t[:], kv_ps[:], istd1_b[:],
                                            op=ALU.mult)
                    qq_t = lay.tile([128, T], MDT, tag="qq")
                    nc.vector.tensor_tensor(qq_t[:], qq_ps[:], istd1_b[:],
                                            op=ALU.mult)

                    # ---- local v -> token-major; stage k|v; gather ----
                    v_loc = lay.tile([128, TC * ATTN], MDT, tag="vloc")
                    for tk in range(TC):
                        tp = ps_x.tile([128, 128], MDT, tag="x")
                        nc.tensor.transpose(
                            tp[:128, :ATTN],
                            vk_t[:ATTN, tk * 128:(tk + 1) * 128], identh[:])
                        nc.vector.tensor_copy(
                            v_loc[:, tk * ATTN:(tk + 1) * ATTN],
                            tp[:128, :ATTN])
                    kv_loc = dram.tile([2 * ATTN * T], MDT, tag="kv_loc")
                    nc.sync.dma_start(
                        kv_loc[0:ATTN * T].rearrange("(a t) -> a t", a=ATTN),
                        vk_t[64:128, :])
                    nc.sync.dma_start(
                        kv_loc[ATTN * T:].rearrange("(p c) -> p c", p=128),
                        v_loc[:])
                    kv_g = dram.tile([GRP, 2 * ATTN * T], MDT, tag="kv_g")
                    nc.gpsimd.collective_compute(
                        "AllGather", ALU.bypass, replica_groups=GROUPS,
                        ins=[kv_loc.opt()], outs=[kv_g.opt()])

                    # ---- attention ----
                    den = ps_rows.tile([128, T], F32, tag="st", name="den")
                    hav = ps_x.tile([128, T], F32, tag="x", name="hav")
                    n_e = 0          # den accumulation counter (20 total)
                    n_av = [0, 0]    # AV chain counters per psum half

                    def av_den(e_t, vsrc, kc):
                        nonlocal n_e
                        half = kc % 2
                        nc.tensor.matmul(den[0:1, :], onesc[:], e_t[:],
                                         start=(n_e == 0), stop=(n_e == 19))
                        n_e += 1
                        nc.tensor.matmul(
                            hav[64 * half:64 * half + 64, :],
                            vsrc, e_t[:],
                            start=(n_av[half] == 0), stop=(n_av[half] == 9))
                        n_av[half] += 1

                    # local quarter first (overlaps the gather)
                    for lk in range(TC):
                        s_ps = ps_mm.tile([128, T], F32, tag="mm")
                        nc.tensor.matmul(s_ps[:],
                                         vk_t[64:128, lk * 128:(lk + 1) * 128],
                                         qq_t[64:128, :], start=True, stop=True)
                        e_t = etp.tile([128, T], MDT, tag="eT")
                        nc.scalar.activation(e_t[:], s_ps[:], AF.Exp,
                                             scale=scale, bias=zbias[:, :1])
                        av_den(e_t, v_loc[:, lk * ATTN:(lk + 1) * ATTN], lk)

                    # gathered full sequence, paired layouts
                    khat = kvp.tile([128, (KC // 2) * 128], MDT, tag="khat")
                    vtm = kvp.tile([128, KC * ATTN], MDT, tag="vtm")
                    for r in range(GRP):
                        src_k = kv_g[r, 0:ATTN * T].rearrange(
                            "(a e two t) -> two a e t", a=ATTN, e=2, two=2,
                            t=128)
                        for two in range(2):
                            dst = khat[64 * two:64 * two + 64,
                                       (2 * r) * 128:(2 * r + 2) * 128]
                            nc.sync.dma_start(
                                dst.rearrange("p (e t) -> p e t", e=2),
                                src_k[two, :, :, :])
                        nc.sync.dma_start(
                            vtm[:, r * TC * ATTN:(r + 1) * TC * ATTN]
                            .rearrange("p (c a) -> p c a", c=TC),
                            kv_g[r, ATTN * T:]
                            .rearrange("(p c a) -> p c a", p=128, c=TC))
                    for pc in range(KC // 2):
                        r = pc // 2
                        sA = ps_mm.tile([128, T], F32, tag="mm")
                        nc.tensor.matmul(sA[:],
                                         khat[0:64, pc * 128:(pc + 1) * 128],
                                         qq_t[0:64, :], start=True, stop=True)
                        sB = ps_mm.tile([128, T], F32, tag="mm")
                        nc.tensor.matmul(sB[:],
                                         khat[64:128, pc * 128:(pc + 1) * 128],
                                         qq_t[64:128, :], start=True, stop=True)
                        eA = etp.tile([128, T], MDT, tag="eT")
                        nc.scalar.activation(eA[:], sA[:], AF.Exp,
                                             scale=scale, bias=mbias_t[:, r:r + 1])
                        eB = etp.tile([128, T], MDT, tag="eT")
                        nc.scalar.activation(eB[:], sB[:], AF.Exp,
                                             scale=scale, bias=mbias_t[:, r:r + 1])
                        kc = 2 * pc
                        av_den(eA, vtm[:, kc * ATTN:(kc + 1) * ATTN], kc)
                        av_den(eB, vtm[:, (kc + 1) * ATTN:(kc + 2) * ATTN],
                               kc + 1)

                    # 1/den (single DVE op), fp16 row, broadcast
                    dsb = rows.tile([1, T], F32, tag="r1")
                    nc.vector.tensor_copy(dsb[:], den[0:1, :])
                    rrow = rows.tile([1, T], F32, tag="r1")
                    nc.vector.reciprocal(rrow[:], dsb[:])
                    rrow16 = rows.tile([1, T], MDT, tag="r1h")
                    nc.vector.tensor_copy(rrow16[:], rrow[:])
                    rb_b = bcast(rrow16, F32, "rb")
                    havB = lay.tile([ATTN, T], F32, tag="havB")
                    nc.scalar.activation(havB[:], hav[64:128, :], AF.Identity,
                                         bias=zbias[:ATTN, :1])
                    headT = lay.tile([ATTN, T], MDT, tag="headT")
                    nc.vector.tensor_tensor(headT[:], hav[0:64, :],
                                            havB[:], op=ALU.add)

                    # ---- proj + residual + LN2 stats (interleaved) ----
                    st = ps_rows.tile([128, T], F32, tag="st")
                    for ec in range(EC):
                        p_ps = ps_mm.tile([128, T], F32, tag="mm")
                        nc.tensor.matmul(p_ps[:], wd_t[:, ec * 128:(ec + 1) * 128],
                                         headT[:], start=True, stop=True)
                        t_sb = sqp.tile([128, T], MDT, tag="tsb")
                        nc.vector.tensor_tensor(t_sb[:], p_ps[:], rb_b[:],
                                                op=ALU.mult)
                        nc.vector.tensor_tensor(hcol(ec), hcol(ec), t_sb[:],
                                                op=ALU.add)
                        emit_stats(st, ec, ec == 0, ec == EC - 1)

                    # ---- LN2 rows; centered h (fp16); FFN unscaled ----
                    nmu16, istd16 = ln_rows(st)
                    nmu2_b = bcast(nmu16, MDT, "m2")
                    hcen = scr.tile([128, EC * T], MDT, tag="hcen")
                    for ec in range(EC):
                        nc.vector.tensor_tensor(
                            hcen[:, ec * T:(ec + 1) * T], hcol(ec), nmu2_b[:],
                            op=ALU.add)
                    istd2_b = bcast(istd16, F32, "i2")
                    for half in range(2):
                        a1 = a1p.tile([128, (HC // 2) * T], MDT, tag="a1")
                        for j in range(HC // 2):
                            hc = half * (HC // 2) + j
                            w1_t = w1p.tile([128, EC * 128], MDT, tag="w1")
                            nc.sync.dma_start(w1_t[:], w1[hc])
                            f_ps = ps_mm.tile([128, T], F32, tag="mm")
                            for ec in range(EC):
                                nc.tensor.matmul(
                                    f_ps[:], w1_t[:, ec * 128:(ec + 1) * 128],
                                    hcen[:, ec * T:(ec + 1) * T],
                                    start=(ec == 0), stop=(ec == EC - 1))
                            nc.scalar.activation(a1[:, j * T:(j + 1) * T], f_ps[:],
                                                 AF.Relu, bias=zbias[:, :1])
                        last_half = half == 1
                        if last_half and layer < STACK - 1:
                            st = ps_rows.tile([128, T], F32, tag="st")
                        for ec in range(EC):
                            w2_t = w2p.tile([128, (HC // 2) * 128], MDT, tag="w2")
                            nc.sync.dma_start(w2_t[:], w2[half, ec])
                            g_ps = ps_mm.tile([128, T], F32, tag="mm")
                            for j in range(HC // 2):
                                nc.tensor.matmul(
                                    g_ps[:], w2_t[:, j * 128:(j + 1) * 128],
                                    a1[:, j * T:(j + 1) * T],
                                    start=(j == 0), stop=(j == HC // 2 - 1))
                            t_sb = sqp.tile([128, T], MDT, tag="tsb")
                            nc.vector.tensor_tensor(t_sb[:], g_ps[:],
                                                    istd2_b[:], op=ALU.mult)
                            nc.vector.tensor_tensor(hcol(ec), hcol(ec), t_sb[:],
                                                    op=ALU.add)
                            if last_half and layer < STACK - 1:
                                emit_stats(st, ec, ec == 0, ec == EC - 1)

        # ======= phase 2: fc, local tokens x full vocab (no collective) =======
        # Groups of GV=4 vocab chunks stay SBUF-resident; within a group the
        # token-chunk stationary is reused across 4 PSUM banks (one per vocab
        # chunk) so LDWEIGHTS amortizes 4x. Output written fp16 (host casts).
        with nc.named_scope("fc"):
            GV = 4
            NVC = VOCAB // VCW
            with tc.tile_pool(name="wfcp", bufs=2) as wfcp, \
                 tc.tile_pool(name="outp", bufs=8) as outp, \
                 tc.tile_pool(name="ps_fc", bufs=2, space="PSUM") as ps_fc:
                for g in range(NVC // GV):
                    wg = wfcp.tile([128, GV * EC * VCW], MDT, tag="wfc")
                    for b in range(GV):
                        nc.sync.dma_start(
                            wg[:, b * EC * VCW:(b + 1) * EC * VCW],
                            wfc[g * GV + b])
                    for tcg in range(TC):
                        # 4 bank-aligned psum slots (512-col padded)
                        o_ps = ps_fc.tile([128, GV, 512], F32, tag="fc")
                        for ec in range(EC):
                            hs = h16[:, ec * T + tcg * 128:
                                     ec * T + (tcg + 1) * 128]
                            for b in range(GV):
                                nc.tensor.matmul(
                                    o_ps[:, b, :VCW], hs,
                                    wg[:, (b * EC + ec) * VCW:
                                       (b * EC + ec + 1) * VCW],
                                    start=(ec == 0), stop=(ec == EC - 1))
                        for b in range(GV):
                            o_sb = outp.tile([128, VCW], MDT, tag="osb")
                            nc.scalar.activation(o_sb[:], o_ps[:, b, :VCW],
                                                 AF.Identity,
                                                 bias=zbias[:, :1])
                            nc.sync.dma_start(
                                out[tcg * 128:(tcg + 1) * 128,
                                    (g * GV + b) * VCW:
                                    (g * GV + b + 1) * VCW], o_sb[:])

    nc.compile()
    return nc


_NC_CACHE = None


def _get_nc():
    global _NC_CACHE
    if _NC_CACHE is None:
        _NC_CACHE = build_nc()
    return _NC_CACHE


def prepare_in_maps(inputs):
    f32 = np.float32
    x = np.asarray(inputs["x"]).reshape(-1).astype(np.int32)
    emb = np.ascontiguousarray(np.asarray(inputs["emb"], f32))
    g1 = np.asarray(inputs["g1"], f32)
    g2 = np.asarray(inputs["g2"], f32)
    # all biases of this model are zero (and beta@W folds are then zero);
    # the kernel relies on that, so assert it here.
    for k in ("bq", "bk", "bv", "bd", "c1", "c2", "bfc", "beta1", "beta2"):
        assert np.abs(np.asarray(inputs[k], f32)).max() == 0.0, f"{k} nonzero"
    # fold LN1 affine into qkv projections; pack [v|k] and [q|q]
    wq_f = (g1[:, None] * np.asarray(inputs["Wq"], f32)).astype(NDT)
    wk_f = (g1[:, None] * np.asarray(inputs["Wk"], f32)).astype(NDT)
    wv_f = (g1[:, None] * np.asarray(inputs["Wv"], f32)).astype(NDT)
    wvk = np.ascontiguousarray(np.concatenate([wv_f, wk_f], axis=1))
    wqq = np.ascontiguousarray(np.concatenate([wq_f, wq_f], axis=1))
    # rank-1 mean-correction rows: colsums of the folded fp16 weights
    svk = np.ascontiguousarray(
        wvk.astype(f32).sum(0, keepdims=True).astype(NDT))
    sqq = np.ascontiguousarray(
        wqq.astype(f32).sum(0, keepdims=True).astype(NDT))
    # tile(head, 16) @ Wd == head @ (sum of the 16 row-blocks of Wd)
    Wd_sum = np.asarray(inputs["Wd"], f32).reshape(N_HEADS, ATTN, EMB).sum(0)
    wd_h = np.ascontiguousarray(Wd_sum.astype(NDT))
    # fold LN2 affine into W1; swizzle to [hc][p][ec*128]
    W1 = np.asarray(inputs["W1"], f32)
    w1_f = (g2[:, None] * W1).astype(NDT)                    # [1024, 4096]
    w1_sw = np.ascontiguousarray(
        w1_f.reshape(EC, 128, HC, 128).transpose(2, 1, 0, 3)
        .reshape(HC, 128, EC * 128))
    # W2 swizzle to [half][ec][p][j*128]
    W2 = np.asarray(inputs["W2"], f32).astype(NDT)           # [4096, 1024]
    w2_sw = np.ascontiguousarray(
        W2.reshape(2, HC // 2, 128, EC, 128).transpose(0, 3, 2, 1, 4)
        .reshape(2, EC, 128, (HC // 2) * 128))
    Wfc = np.asarray(inputs["Wfc"], f32)

    NVC = VOCAB // VCW
    wfc_sw = np.ascontiguousarray(
        Wfc.astype(NDT).reshape(EC, 128, NVC, VCW).transpose(2, 1, 0, 3)
        .reshape(NVC, 128, EC * VCW))
    in_maps = []
    for c in range(N_CORES):
        mb = np.zeros((128, GRP), np.float32)
        mb[:, c % GRP] = -1e4
        in_maps.append(dict(
            emb=emb,
            xi=np.ascontiguousarray(x[c * T:(c + 1) * T, None]),
            wvk=wvk, wqq=wqq, svk=svk, sqq=sqq,
            wd=wd_h, w1=w1_sw, w2=w2_sw,
            wfc=wfc_sw, mbias=mb,
        ))
    return in_maps


def kernel(**inputs) -> np.ndarray:
    nc = _get_nc()
    in_maps = prepare_in_maps(inputs)
    r = run_bass_kernel_spmd(nc, in_maps, core_ids=list(range(N_CORES)))
    logits = np.concatenate([r.results[c]["out"] for c in range(N_CORES)], axis=0)
    return logits.reshape(BATCH, SEQ, VOCAB).astype(np.float32)
